# revision 1
# baseline (speedup 1.0000x reference)
"""Trainium2 Bass kernel for nn_LinearAttention (B=8,S=4096,F=256,I=512,D=4,K=7,V=256).

Sharding: data-parallel over batch — one sample per NeuronCore (8 cores).
Per-core layout is channel-major [C, S]. Per depth:
  Pass A: h = w0 @ x1 (bf16 MM) -> mish -> linattn -> v1 (bf16, SBUF-resident)
  Pass B: g = causal-conv7(v1) (bf16 MM, 28-MM PSUM groups) -> mish -> linattn
          -> v2 -> w2 @ v2 accumulated into out PSUM -> x1' = x0 + out
Embedding gather = one-hot matmul (fp32, exact). Final 1x1 conv in fp32.
mish(x) = x * tanh(ln(1 + exp(x))) via ACT Exp/Ln(bias=1)/Tanh LUTs,
batched per function to limit ACT table switches.
cumsum via VectorE tensor_tensor_scan (fp32 state), chained across blocks.
"""
import sys
sys.path.insert(0, '/opt/trn_rl_repo')
import numpy as np
import ml_dtypes
from contextlib import ExitStack

import concourse.bass as bass
import concourse.bacc as bacc
import concourse.tile as tile
import concourse.mybir as mybir
from concourse.bass_utils import run_bass_kernel_spmd

F32 = mybir.dt.float32
BF16 = mybir.dt.bfloat16
AF = mybir.ActivationFunctionType
ALU = mybir.AluOpType

B, S, F, I, D, KK, V = 8, 4096, 256, 512, 4, 7, 256
T = 512          # seq block
NB = S // T      # 8 blocks
PAD = 8          # left pad cols in v1 buffer (first 6 used)


def build_program(reps=1):
    nc = bacc.Bacc("TRN2", target_bir_lowering=False, debug=False, num_devices=8)

    inp_b = nc.dram_tensor("inp_b", [128, S], F32, kind="ExternalInput").ap()
    emb_d = nc.dram_tensor("emb", [V, 2 * F], F32, kind="ExternalInput").ap()
    w0t_d = nc.dram_tensor("w0t", [D, F, 3 * I], BF16, kind="ExternalInput").ap()
    w1t_d = nc.dram_tensor("w1t", [D, KK, I, 3 * I], BF16, kind="ExternalInput").ap()
    w2t_d = nc.dram_tensor("w2t", [D, I, F], BF16, kind="ExternalInput").ap()
    outwt_d = nc.dram_tensor("outwt", [2 * F, V], F32, kind="ExternalInput").ap()
    outb_d = nc.dram_tensor("outb", [V, 1], F32, kind="ExternalInput").ap()
    iota_d = nc.dram_tensor("iota", [128, 2], F32, kind="ExternalInput").ap()
    recip_d = nc.dram_tensor("recip", [128, S], F32, kind="ExternalInput").ap()

    out_d = nc.dram_tensor("out", [V, S], F32, kind="ExternalOutput").ap()

    # X[0]=x0_init, X[1]=x1_init, X[d+2] = X[d] + cell(X[d+1])
    X = [nc.dram_tensor(f"X{i}", [F, S], F32).ap() for i in range(D + 2)]

    with tile.TileContext(nc) as tc, ExitStack() as ctx:
        # ---- persistent pools ----
        cpool = ctx.enter_context(tc.tile_pool(name="const", bufs=1))
        v1pool = ctx.enter_context(tc.tile_pool(name="v1", bufs=1))
        psH = ctx.enter_context(tc.tile_pool(name="psH", bufs=6, space="PSUM"))
        psO = ctx.enter_context(tc.tile_pool(name="psO", bufs=1, space="PSUM"))

        embsb = [cpool.tile([128, 2 * F], F32, name=f"emb{vt}", tag=f"emb{vt}") for vt in range(2)]
        for vt in range(2):
            nc.sync.dma_start(embsb[vt][:], emb_d[vt * 128:(vt + 1) * 128, :])
        outwsb = [cpool.tile([128, V], F32, name=f"ow{kt}", tag=f"ow{kt}") for kt in range(4)]
        for kt in range(4):
            nc.sync.dma_start(outwsb[kt][:], outwt_d[kt * 128:(kt + 1) * 128, :])
        outbsb = [cpool.tile([128, 1], F32, name=f"ob{mo}", tag=f"ob{mo}") for mo in range(2)]
        for mo in range(2):
            nc.sync.dma_start(outbsb[mo][:], outb_d[mo * 128:(mo + 1) * 128, :])
        iotasb = cpool.tile([128, 2], F32, name="iota", tag="iota")
        nc.sync.dma_start(iotasb[:], iota_d)
        ones = cpool.tile([128, T], F32, name="ones", tag="ones")
        nc.vector.memset(ones[:], 1.0)

        v1sb = [v1pool.tile([128, S + PAD], BF16, name=f"v1_{i}", tag=f"v1_{i}") for i in range(4)]
        for i in range(4):
            nc.vector.memset(v1sb[i][:, 0:PAD], 0.0)

        # ---- embedding gather via one-hot matmul (fp32, exact) ----
        # ---- main depth stack ----
        wpool = ctx.enter_context(tc.tile_pool(name="w", bufs=1))
        work = ctx.enter_context(tc.tile_pool(name="work", bufs=2))
        mishp = ctx.enter_context(tc.tile_pool(name="mish", bufs=1))
        cump = ctx.enter_context(tc.tile_pool(name="cum", bufs=2))

        def linattn_tail(hps, th, carry, b, recw, out_ap, tag, i):
            """mish muls (in-place into th tiles) + scan + normalize."""
            for q in range(3):
                nc.vector.tensor_mul(th[q][:], hps[q][:], th[q][:])
            cum = cump.tile([128, T], F32, name=f"{tag}cum", tag=f"{tag}cum")
            init = 0.0 if b == 0 else carry[:, 0:1]
            nc.vector.tensor_tensor_scan(
                cum[:], ones[:], th[0][:], init, ALU.mult, ALU.add)
            nc.vector.tensor_copy(carry[:, 0:1], cum[:, T - 1:T])
            nc.vector.tensor_mul(th[0][:], cum[:], recw[:])
            nc.vector.tensor_mul(th[0][:], th[0][:], th[1][:])
            nc.vector.tensor_tensor(out_ap, th[0][:], th[2][:], ALU.add)

        def mish_acts(hps, tag):
            """Batched Exp -> Ln(+1) -> Tanh on 3 chunks; returns tanh tiles."""
            eu = [mishp.tile([128, T], F32, name=f"{tag}eu{q}", tag=f"{tag}eu{q}") for q in range(3)]
            for q in range(3):
                nc.scalar.activation(eu[q][:], hps[q][:], AF.Exp)
            for q in range(3):  # in-place Ln(1+u)
                nc.scalar.activation(eu[q][:], eu[q][:], AF.Ln, bias=1.0)
            th = eu
            for q in range(3):
                nc.scalar.activation(th[q][:], th[q][:], AF.Tanh)
            return th

        for _rep in range(reps):
          with tc.tile_pool(name="embed", bufs=2) as epool:
              for b in range(NB):
                  inpblk = epool.tile([128, T], F32, name="inpblk", tag="inpblk")
                  nc.sync.dma_start(inpblk[:], inp_b[:, b * T:(b + 1) * T])
                  oh = []
                  for vt in range(2):
                      t = epool.tile([128, T], F32, name=f"oh{vt}", tag=f"oh{vt}")
                      nc.vector.tensor_scalar(
                          t[:], inpblk[:],
                          iotasb[:, vt:vt + 1], None, ALU.is_equal)
                      oh.append(t)
                  for j in range(4):
                      ps = psH.tile([128, T], F32, name="ps", tag="ps")
                      for vt in range(2):
                          nc.tensor.matmul(
                              ps[:], embsb[vt][:, j * 128:(j + 1) * 128], oh[vt][:],
                              start=(vt == 0), stop=(vt == 1))
                      xe = epool.tile([128, T], F32, name="xe", tag="xe", bufs=1)
                      nc.vector.tensor_copy(xe[:], ps[:])
                      dst = X[j // 2]
                      nc.sync.dma_start(
                          dst[(j % 2) * 128:(j % 2 + 1) * 128, b * T:(b + 1) * T], xe[:])

          for d in range(D):
              w0sb = [wpool.tile([128, 3 * I], BF16, name=f"w0_{kt}", tag=f"w0_{kt}") for kt in range(2)]
              for kt in range(2):
                  nc.sync.dma_start(w0sb[kt][:], w0t_d[d, kt * 128:(kt + 1) * 128, :])
              w1sb = [[wpool.tile([128, 3 * I], BF16, name=f"w1_{k}_{i}", tag=f"w1_{k}_{i}")
                       for i in range(4)] for k in range(KK)]
              for k in range(KK):
                  for i in range(4):
                      nc.sync.dma_start(
                          w1sb[k][i][:], w1t_d[d, k, i * 128:(i + 1) * 128, :])
              w2sb = [wpool.tile([128, F], BF16, name=f"w2_{i}", tag=f"w2_{i}") for i in range(4)]
              for i in range(4):
                  nc.sync.dma_start(w2sb[i][:], w2t_d[d, i * 128:(i + 1) * 128, :])

              # ---- Pass A: x1 -> v1 ----
              carA = [cpool.tile([128, 1], F32, name=f"carA{i}", tag=f"carA{i}")
                      for i in range(4)] if d == 0 else carA
              for b in range(NB):
                  x1blk = [work.tile([128, T], F32, name=f"x1_{kt}", tag=f"x1_{kt}") for kt in range(2)]
                  for kt in range(2):
                      nc.sync.dma_start(
                          x1blk[kt][:], X[d + 1][kt * 128:(kt + 1) * 128, b * T:(b + 1) * T])
                  x1b = [work.tile([128, T], BF16, name=f"x1b_{kt}", tag=f"x1b_{kt}") for kt in range(2)]
                  for kt in range(2):
                      nc.vector.tensor_copy(x1b[kt][:], x1blk[kt][:])
                  reca = work.tile([128, T], F32, name="reca", tag="reca")
                  nc.sync.dma_start(reca[:], recip_d[:, b * T:(b + 1) * T])
                  for i in range(4):
                      hps = []
                      for q in range(3):
                          m = q * 4 + i
                          ps = psH.tile([128, T], F32, name="ps", tag="ps")
                          for kt in range(2):
                              nc.tensor.matmul(
                                  ps[:], w0sb[kt][:, m * 128:(m + 1) * 128], x1b[kt][:],
                                  start=(kt == 0), stop=(kt == 1))
                          hps.append(ps)
                      th = mish_acts(hps, "A")
                      linattn_tail(
                          hps, th, carA[i], b, reca,
                          v1sb[i][:, PAD + b * T: PAD + b * T + T], "A", i)
                      # out slice is v1sb[i][:, PAD + b*T : PAD + b*T + T]

              # ---- Pass B: v1 -> conv -> v2 -> out; x1' = x0 + out ----
              carB = [cpool.tile([128, 1], F32, name=f"carB{i}", tag=f"carB{i}")
                      for i in range(4)] if d == 0 else carB
              for b in range(NB):
                  recb = work.tile([128, T], F32, name="recb", tag="recb")
                  nc.sync.dma_start(recb[:], recip_d[:, b * T:(b + 1) * T])
                  x0blk = [work.tile([128, T], F32, name=f"x0_{mo}", tag=f"x0_{mo}") for mo in range(2)]
                  for mo in range(2):
                      nc.sync.dma_start(
                          x0blk[mo][:], X[d][mo * 128:(mo + 1) * 128, b * T:(b + 1) * T])
                  outps = [psO.tile([128, T], F32, name=f"ops{mo}", tag=f"ops{mo}") for mo in range(2)]
                  base = PAD - 6 + b * T  # rhs col for tap k=0
                  for i in range(4):
                      hps = []
                      for q in range(3):
                          m = q * 4 + i
                          ps = psH.tile([128, T], F32, name="ps", tag="ps")
                          first = True
                          for k in range(KK):
                              for kt in range(4):
                                  nc.tensor.matmul(
                                      ps[:], w1sb[k][kt][:, m * 128:(m + 1) * 128],
                                      v1sb[kt][:, base + k: base + k + T],
                                      start=first, stop=(k == KK - 1 and kt == 3))
                                  first = False
                          hps.append(ps)
                      th = mish_acts(hps, "B")
                      v2c = work.tile([128, T], BF16, name="v2c", tag="v2c")
                      linattn_tail(hps, th, carB[i], b, recb, v2c[:], "B", i)
                      for mo in range(2):
                          nc.tensor.matmul(
                              outps[mo][:], w2sb[i][:, mo * 128:(mo + 1) * 128], v2c[:],
                              start=(i == 0), stop=(i == 3), skip_group_check=True)
                  for mo in range(2):
                      nx = work.tile([128, T], F32, name=f"nx{mo}", tag=f"nx{mo}")
                      nc.vector.tensor_tensor(nx[:], outps[mo][:], x0blk[mo][:], ALU.add)
                      nc.sync.dma_start(
                          X[d + 2][mo * 128:(mo + 1) * 128, b * T:(b + 1) * T], nx[:])

          # ---- final 1x1 conv (fp32, exact): out = outwT.T @ [X4; X5] + b ----
          for b in range(NB):
              _tg = ["x1_0", "x1_1", "x0_0", "x0_1"]
              xc = [work.tile([128, T], F32, name=_tg[kt], tag=_tg[kt]) for kt in range(4)]
              for kt in range(4):
                  src = X[4 + kt // 2]
                  nc.sync.dma_start(
                      xc[kt][:], src[(kt % 2) * 128:(kt % 2 + 1) * 128, b * T:(b + 1) * T])
              for mo in range(2):
                  ps = psO.tile([128, T], F32, name=f"ops{mo}", tag=f"ops{mo}")
                  for kt in range(4):
                      nc.tensor.matmul(
                          ps[:], outwsb[kt][:, mo * 128:(mo + 1) * 128], xc[kt][:],
                          start=(kt == 0), stop=(kt == 3))
                  ob = work.tile([128, T], F32, name=f"nx{mo}", tag=f"nx{mo}")
                  nc.vector.tensor_scalar(
                      ob[:], ps[:], outbsb[mo][:, 0:1], None, ALU.add)
                  nc.sync.dma_start(
                      out_d[mo * 128:(mo + 1) * 128, b * T:(b + 1) * T], ob[:])

    nc.compile()
    return nc


_NC = None


def _prep_inputs(inp, emb, w0, w1, w2, out_w, out_b):
    inp = np.asarray(inp).astype(np.float32)          # [B, S]
    emb = np.ascontiguousarray(np.asarray(emb), dtype=np.float32)
    w0t = np.ascontiguousarray(
        np.asarray(w0)[:, :, :, 0].transpose(0, 2, 1)).astype(ml_dtypes.bfloat16)
    w1t = np.ascontiguousarray(
        np.asarray(w1).transpose(0, 3, 2, 1)).astype(ml_dtypes.bfloat16)
    w2t = np.ascontiguousarray(
        np.asarray(w2)[:, :, :, 0].transpose(0, 2, 1)).astype(ml_dtypes.bfloat16)
    outwt = np.ascontiguousarray(np.asarray(out_w)[:, :, 0].T).astype(np.float32)
    outb = np.asarray(out_b).astype(np.float32).reshape(V, 1)
    iota = np.stack([np.arange(128, dtype=np.float32),
                     np.arange(128, 256, dtype=np.float32)], axis=1)
    recip = np.broadcast_to(
        (1.0 / np.arange(1, S + 1, dtype=np.float32))[None, :], (128, S)).copy()
    in_maps = []
    for c in range(B):
        in_maps.append({
            "inp_b": np.broadcast_to(inp[c][None, :], (128, S)).copy(),
            "emb": emb, "w0t": w0t, "w1t": w1t, "w2t": w2t,
            "outwt": outwt, "outb": outb, "iota": iota, "recip": recip,
        })
    return in_maps


def _run(in_maps, trace=False):
    global _NC
    if _NC is None:
        _NC = build_program()
    res = run_bass_kernel_spmd(_NC, in_maps, list(range(8)), trace=trace)
    out = np.stack([res.results[c]["out"] for c in range(B)], axis=0)
    return out.astype(np.float32), res


def kernel(inp, emb, w0, w1, w2, out_w, out_b):
    in_maps = _prep_inputs(inp, emb, w0, w1, w2, out_w, out_b)
    out, _ = _run(in_maps, trace=False)
    return out


def run_traced(inp, emb, w0, w1, w2, out_w, out_b):
    in_maps = _prep_inputs(inp, emb, w0, w1, w2, out_w, out_b)
    return _run(in_maps, trace=True)

